# revision 26
# baseline (speedup 1.0000x reference)
"""v4.1 feature kernel.

Measured-model driven layout:
- DVE (V) does all word algebra; ~0.52 ns/e TS, ~1.04 ns/e TT/STT, +~200ns/op.
- GpSimd unused for compute: Pool has no bitwise/TS support, and its memsets
  contend with DVE for SBUF ports (measured 3x slowdown on concurrent ops).
- All guard memsets run on V up front, inside the input-DMA idle window.
- Act does the strided u16->f32 expansions for the line group (2x stride
  penalty measured, still worth it: runs while V computes conn) plus the
  doubles broadcasts; V converts the conn group at stream end (V idle then).
- Extraction is j-major over all 32 boards (8 TS ops per 3-lane group).
- Conn group extracted+converted per 16-board half to shorten the DMA tail.
- TMP (line temps) and CV (conn pyramids) share one scratch allocation
  (disjoint lifetimes); mkL is reused for the conn extraction.
"""
import numpy as np

import concourse.bass as bass
import concourse.bacc as bacc
import concourse.mybir as mybir
import concourse.tile as tile

Alu = mybir.AluOpType
Act = mybir.ActivationFunctionType
DT = mybir.dt

P = 128
NB = 32
CB = 8             # boards per output chunk
NCORES = 8
BPC = P * NB
PAD = 18
R0 = 5
ROWS = slice(R0, R0 + 8)
SEG = 0x00FF00FF
PADT = 12
R0T = 2

DIRS = ((0, 1), (1, 0), (1, 1), (1, -1))

NAMES = ("t", "u", "a", "w", "b", "y", "q", "l2", "m3", "r1", "c",
         "i1", "l3", "lb", "d0", "d1", "d", "j1", "md", "o3", "rb",
         "x", "e", "g1", "g2", "r3", "c_l2", "c_l3", "c_r3")
NLANES = len(NAMES)
SCR_WORDS = NLANES * NB * PADT           # TMP scratch, u32 per partition
CV_WORDS = 2 * 5 * NB * PAD
assert CV_WORDS <= SCR_WORDS


def _stt_raw(eng, out, in0, imm, in1, op0, op1, imm_dt=DT.uint32):
    outs = [eng.lower_ap(out)]
    return eng.add_instruction(
        mybir.InstTensorScalarPtr(
            name=eng.bass.get_next_instruction_name(),
            is_scalar_tensor_tensor=True,
            op0=op0, op1=op1,
            ins=[eng.lower_ap(in0),
                 mybir.ImmediateValue(dtype=imm_dt, value=imm),
                 eng.lower_ap(in1)],
            outs=outs,
        )
    )


def _stt(eng, out, in0, sh, op1, in1):
    if sh > 0:
        _stt_raw(eng, out, in0, sh, in1, Alu.logical_shift_left, op1)
    elif sh < 0:
        _stt_raw(eng, out, in0, -sh, in1, Alu.logical_shift_right, op1)
    else:
        eng.tensor_tensor(out, in0, in1, op1)


def feature_kernel(tc, out_d, state_d):
    nc = tc.nc
    V, A = nc.vector, nc.scalar

    state_v = state_d.rearrange("(p n) c -> p n c", p=P)
    out_v = out_d.rearrange("(p n) c -> p n c", p=P)

    with (
        tc.tile_pool(name="main", bufs=1) as pool,
        tc.tile_pool(name="exp", bufs=4) as epool,
    ):
        myR = pool.tile([P, NB, 8], DT.uint32, name="myR")
        opR = pool.tile([P, NB, 8], DT.uint32, name="opR")
        P4 = pool.tile([P, 4, NB, PAD], DT.uint32, name="P4")
        Rg = pool.tile([P, 6, NB, 8], DT.uint32, name="Rg")
        dge = pool.tile([P, 4, NB], DT.float32, name="dge")
        lbm = pool.tile([P, NB, 8], DT.uint32, name="lbm")
        rbm = pool.tile([P, NB, 8], DT.uint32, name="rbm")
        mkL = pool.tile([P, 3, NB, 64], DT.uint32, name="mkL")
        w1 = pool.tile([P, 3, NB, 8], DT.uint32, name="w1")
        w2 = pool.tile([P, 3, NB, 8], DT.uint32, name="w2")
        cs23 = pool.tile([P, 4, NB], DT.uint32, name="cs23")
        scratch = pool.tile([P, SCR_WORDS], DT.uint32, name="scratch")
        AV = pool.tile([P, 12, NB, 8], DT.uint32, name="AV")
        cx1 = pool.tile([P, NB, 8], DT.uint32, name="cx1")
        cx2 = pool.tile([P, NB, 8], DT.uint32, name="cx2")

        TMP = scratch.rearrange("p (l n c) -> p l n c", l=NLANES, n=NB)
        CV = scratch[:, 0:CV_WORDS].rearrange(
            "p (w f n c) -> p w f n c", w=2, f=5, n=NB)
        nix = {n: i for i, n in enumerate(NAMES)}

        with tc.tile_pool(name="pre", bufs=1) as ppre:
            s8 = ppre.tile([P, NB, 64], DT.int8, name="s8")
            nc.sync.dma_start(s8[:], state_v)
            mo = ppre.tile([P, NB, 2, 64], DT.bfloat16, name="mo")
            pk1 = ppre.tile([P, NB, 8, 4], DT.float32, name="pk1")
            pk2 = ppre.tile([P, NB, 8, 2], DT.float32, name="pk2")

            A.activation(mo[:, :, 0], s8[:], Act.Relu)
            A.activation(mo[:, :, 1], s8[:], Act.Relu, 0.0, -1.0)
            V.memset(P4[:, :, :, 0:R0], 0)
            V.memset(P4[:, :, :, R0 + 8:PAD], 0)
            V.memset(lbm[:], 0)
            V.memset(lbm[:, :, 0:1], SEG)
            V.memset(rbm[:], 0)
            V.memset(rbm[:, :, 3:8], SEG)
            nc.sync.dma_start(
                out_v[:, :, 0:128],
                mo.rearrange("p n c x -> p n (c x)"))

            def pack(dst_ap, srcf):
                v = srcf.rearrange("p n (r j2 t) -> p n r j2 t", t=2, j2=4)
                a1, b1 = v[:, :, :, :, 1], v[:, :, :, :, 0]
                V.scalar_tensor_tensor(pk1[:], a1, 2.0, b1, op0=Alu.mult,
                                       op1=Alu.add)
                ww2 = pk1.rearrange("p n r (k t) -> p n r k t", t=2)
                a2, b2 = ww2[:, :, :, :, 1], ww2[:, :, :, :, 0]
                V.scalar_tensor_tensor(pk2[:], a2, 4.0, b2, op0=Alu.mult,
                                       op1=Alu.add)
                a3, b3 = pk2[:, :, :, 1], pk2[:, :, :, 0]
                V.scalar_tensor_tensor(dst_ap, a3, 16.0, b3, op0=Alu.mult,
                                       op1=Alu.add)

            pack(myR[:], mo[:, :, 0])
            pack(opR[:], mo[:, :, 1])

        # ---------- planes ----------
        Ad, Bd, Ed, Nd = (P4[:, i] for i in range(4))
        _stt(V, Ad[:, :, ROWS], opR[:], 16, Alu.bitwise_or, myR[:])
        _stt(V, Bd[:, :, ROWS], myR[:], 16, Alu.bitwise_or, opR[:])
        V.tensor_tensor(Ed[:, :, ROWS], Ad[:, :, ROWS], Bd[:, :, ROWS],
                        Alu.bitwise_or)
        V.tensor_scalar(Ed[:, :, ROWS], Ed[:, :, ROWS], SEG, None,
                        Alu.bitwise_xor)
        V.tensor_scalar(Nd[:, :, ROWS], Ad[:, :, ROWS], SEG, None,
                        Alu.bitwise_xor)

        V.memset(TMP[:, :, :, 0:R0T], 0)
        V.memset(TMP[:, :, :, R0T + 8:PADT], 0)

        def TT(n, k=0):
            return TMP[:, nix[n], :, R0T + k:R0T + 8 + k]

        # ---------- line features: rows (3 sub-chains zipped) ----------
        me, op, em, nm = (x[:, :, ROWS] for x in (Ad, Bd, Ed, Nd))
        T = TT
        _stt(V, T("t"), me, -1, Alu.bitwise_and, me)
        _stt(V, T("u"), em, -1, Alu.bitwise_and, em)
        V.tensor_scalar(T("lb"), op, 1, 0x00010001,
                        op0=Alu.logical_shift_left, op1=Alu.bitwise_or)
        _stt(V, T("m3"), me, -2, Alu.bitwise_and, T("t"))
        _stt(V, T("a"), T("u"), -2, Alu.bitwise_and, T("t"))
        _stt(V, T("w"), em, -3, Alu.bitwise_and, em)
        V.tensor_scalar(T("rb"), op, 5, 0x00F800F8,
                        op0=Alu.logical_shift_right, op1=Alu.bitwise_or)
        _stt(V, T("r1"), em, -4, Alu.bitwise_and, em)
        _stt(V, T("b"), T("t"), -1, Alu.bitwise_and, T("w"))
        _stt(V, T("d0"), em, -3, Alu.bitwise_and, T("m3"))
        _stt(V, T("c"), T("m3"), -1, Alu.bitwise_and, T("r1"))
        _stt(V, T("y"), T("b"), 1, Alu.bitwise_or, T("b"))
        _stt(V, T("d1"), nm, -4, Alu.bitwise_and, T("d0"))
        _stt(V, T("i1"), T("c"), 1, Alu.bitwise_or, T("c"))
        _stt(V, T("o3"), T("m3"), -1, Alu.bitwise_and, nm)
        V.tensor_tensor(T("q"), T("a"), T("y"), Alu.bitwise_or)
        V.tensor_tensor(T("d"), T("d1"), T("lb"), Alu.bitwise_and)
        _stt(V, T("l3"), T("i1"), 1, Alu.bitwise_or, T("c"))
        _stt(V, T("o3"), nm, -4, Alu.bitwise_and, T("o3"))
        _stt(V, T("l2"), T("q"), 1, Alu.bitwise_or, T("a"))
        _stt(V, T("j1"), T("d"), 1, Alu.bitwise_or, T("d"))
        V.tensor_tensor(T("x"), T("lb"), T("rb"), Alu.bitwise_xor)
        _stt(V, T("md"), T("d"), 2, Alu.bitwise_or, T("j1"))
        V.tensor_tensor(T("e"), T("o3"), T("x"), Alu.bitwise_and)
        _stt(V, T("g1"), T("e"), 1, Alu.bitwise_or, T("e"))
        _stt(V, T("g2"), T("g1"), 1, Alu.bitwise_or, T("e"))
        _stt(V, T("r3"), T("g2"), 1, Alu.bitwise_or, T("md"))

        # ---------- line features: cols (offset addressing, zipped) ----
        def dn(x, k):
            return x[:, :, R0 + k:R0 + 8 + k]

        def Tc(n, k=0):
            nm2 = "c_" + n if n in ("l2", "l3", "r3") else n
            return TT(nm2, k)

        def MV(x, k=0):
            return x[:, :, R0 + k:R0 + 8 + k]

        me4, op4, em4, nm4 = Ad, Bd, Ed, Nd
        V.tensor_tensor(Tc("t"), MV(me4), dn(me4, 1), Alu.bitwise_and)
        V.tensor_tensor(Tc("u"), MV(em4), dn(em4, 1), Alu.bitwise_and)
        V.tensor_tensor(Tc("lb"), MV(op4, -1), lbm[:], Alu.bitwise_or)
        V.tensor_tensor(Tc("m3"), Tc("t"), dn(me4, 2), Alu.bitwise_and)
        V.tensor_tensor(Tc("a"), Tc("t"), Tc("u", 2), Alu.bitwise_and)
        V.tensor_tensor(Tc("w"), MV(em4), dn(em4, 3), Alu.bitwise_and)
        V.tensor_tensor(Tc("rb"), MV(op4, 5), rbm[:], Alu.bitwise_or)
        V.tensor_tensor(Tc("r1"), MV(em4), dn(em4, 4), Alu.bitwise_and)
        V.tensor_tensor(Tc("b"), Tc("w"), Tc("t", 1), Alu.bitwise_and)
        V.tensor_tensor(Tc("d0"), Tc("m3"), dn(em4, 3), Alu.bitwise_and)
        V.tensor_tensor(Tc("c"), Tc("r1"), Tc("m3", 1), Alu.bitwise_and)
        V.tensor_tensor(Tc("y"), Tc("b"), Tc("b", -1), Alu.bitwise_or)
        V.tensor_tensor(Tc("d1"), Tc("d0"), dn(nm4, 4), Alu.bitwise_and)
        V.tensor_tensor(Tc("i1"), Tc("c"), Tc("c", -1), Alu.bitwise_or)
        V.tensor_tensor(Tc("o3"), Tc("m3", 1), MV(nm4), Alu.bitwise_and)
        V.tensor_tensor(Tc("q"), Tc("a"), Tc("y"), Alu.bitwise_or)
        V.tensor_tensor(Tc("d"), Tc("d1"), Tc("lb"), Alu.bitwise_and)
        V.tensor_tensor(Tc("l3"), Tc("c"), Tc("i1", -1), Alu.bitwise_or)
        V.tensor_tensor(Tc("o3"), Tc("o3"), dn(nm4, 4), Alu.bitwise_and)
        V.tensor_tensor(Tc("l2"), Tc("a"), Tc("q", -1), Alu.bitwise_or)
        V.tensor_tensor(Tc("j1"), Tc("d"), Tc("d", -1), Alu.bitwise_or)
        V.tensor_tensor(Tc("x"), Tc("lb"), Tc("rb"), Alu.bitwise_xor)
        V.tensor_tensor(Tc("md"), Tc("j1"), Tc("d", -2), Alu.bitwise_or)
        V.tensor_tensor(Tc("e"), Tc("o3"), Tc("x"), Alu.bitwise_and)
        V.tensor_tensor(Tc("g1"), Tc("e"), Tc("e", -1), Alu.bitwise_or)
        V.tensor_tensor(Tc("g2"), Tc("e"), Tc("g1", -1), Alu.bitwise_or)
        V.tensor_tensor(Tc("r3"), Tc("md"), Tc("g2", -1), Alu.bitwise_or)

        V.tensor_tensor(Rg[:, 3], TT("l2"), TT("c_l2"), Alu.bitwise_or)
        _stt(V, Rg[:, 4], TT("l3"), 1, Alu.bitwise_or, TT("c_l3", -1))
        V.tensor_tensor(Rg[:, 5], TT("r3"), TT("c_r3"), Alu.bitwise_or)

        # ---------- doubles (SWAR popcount, all V) ----------
        V.tensor_scalar(w1[:], Rg[:, 3:6], 1, 0x00550055,
                        op0=Alu.logical_shift_right, op1=Alu.bitwise_and)
        V.tensor_tensor(w1[:], Rg[:, 3:6], w1[:], Alu.subtract)
        V.tensor_scalar(w2[:], w1[:], 2, 0x00330033,
                        op0=Alu.logical_shift_right, op1=Alu.bitwise_and)
        V.tensor_scalar(w1[:], w1[:], 0x00330033, None, Alu.bitwise_and)
        V.tensor_tensor(w1[:], w1[:], w2[:], Alu.add)
        V.tensor_scalar(w2[:], w1[:], 4, None, Alu.logical_shift_right)
        V.tensor_tensor(w1[:], w1[:], w2[:], Alu.add)
        V.tensor_scalar(w1[:], w1[:], 0x000F000F, None, Alu.bitwise_and)
        V.tensor_tensor(w1[:, :, :, 0:4], w1[:, :, :, 0:4],
                        w1[:, :, :, 4:8], Alu.add)
        V.tensor_tensor(w1[:, :, :, 0:2], w1[:, :, :, 0:2],
                        w1[:, :, :, 2:4], Alu.add)
        V.tensor_tensor(cs23[:, 0:3], w1[:, :, :, 0], w1[:, :, :, 1],
                        Alu.add)
        V.tensor_tensor(cs23[:, 3], cs23[:, 1], cs23[:, 2], Alu.add)
        V.tensor_scalar(cs23[:, 1], cs23[:, 0], 0xFF, None, Alu.bitwise_and)
        V.tensor_scalar(cs23[:, 2], cs23[:, 3], 0xFF, None, Alu.bitwise_and)
        V.tensor_scalar(cs23[:, 0], cs23[:, 0], 16, None,
                        Alu.logical_shift_right)
        V.tensor_scalar(cs23[:, 3], cs23[:, 3], 16, None,
                        Alu.logical_shift_right)
        V.tensor_scalar(dge[:, 0], cs23[:, 1], 2, None, Alu.is_ge)
        V.tensor_scalar(dge[:, 1], cs23[:, 2], 2, None, Alu.is_ge)
        V.tensor_scalar(dge[:, 2], cs23[:, 0], 2, None, Alu.is_ge)
        V.tensor_scalar(dge[:, 3], cs23[:, 3], 2, None, Alu.is_ge)

        # ---------- line-group extraction (j-major, all boards) ----------
        for j in range(8):
            V.tensor_scalar(mkL[:, :, :, j::8], Rg[:, 3:6], j, 0x00010001,
                            op0=Alu.logical_shift_right, op1=Alu.bitwise_and)
        mkL16 = mkL[:].bitcast(DT.uint16)   # [P, 3, NB, 128]

        # ---------- line-group conversion + DMA ----------
        # chunks 0-2 convert on Act (runs while V computes conn);
        # chunk 3 converts on V (then V moves to conn).
        line_outts = []
        for ci, n0 in enumerate(range(0, NB, CB)):
            hs = slice(n0, n0 + CB)
            outt = epool.tile([P, CB, 10, 64], DT.bfloat16, name="outt")
            line_outts.append((hs, outt))
            A.activation(
                outt[:, :, 0:3],
                mkL16[:, :, hs, 0::2].rearrange("p c n x -> p n c x"),
                Act.Copy)
            A.activation(
                outt[:, :, 5:8],
                mkL16[:, :, hs, 1::2].rearrange("p c n x -> p n c x"),
                Act.Copy)
            A.activation(
                outt[:, :, 3:5],
                dge[:, 0:2, hs].rearrange("p d n -> p n d")[:, :, :, None]
                .broadcast_to((P, CB, 2, 64)), Act.Copy)
            A.activation(
                outt[:, :, 8:10],
                dge[:, 2:4, hs].rearrange("p d n -> p n d")[:, :, :, None]
                .broadcast_to((P, CB, 2, 64)), Act.Copy)
            leng = nc.gpsimd if ci % 2 == 1 else nc.sync
            leng.dma_start(
                out_v[:, hs, 512:1152],
                outt.rearrange("p b c x -> p b (c x)"))

        # ---------- connectivity (2-dir interleaved chains) ----------
        V.memset(CV[:, :, :, :, R0 - 1:R0], 0)
        V.memset(CV[:, :, :, :, R0 + 8:R0 + 10], 0)

        mv = Ad[:, :, ROWS]

        def conn_ops(w, k):
            di, dj = DIRS[k]
            d2, d3, d4, t3, t4 = (CV[:, w, i] for i in range(5))

            def fwd(t):
                return t[:, :, R0 - di:R0 + 8 - di]

            def bwd(t, m=1):
                return t[:, :, R0 + m * di:R0 + 8 + m * di]

            a2, a3, a4 = (AV[:, 3 * k + i] for i in range(3))
            yield lambda: _stt(V, d2[:, :, ROWS], fwd(Ad), dj,
                               Alu.bitwise_and, mv)
            yield lambda: _stt(V, d3[:, :, ROWS], fwd(d2), dj,
                               Alu.bitwise_and, d2[:, :, ROWS])
            yield lambda: _stt(V, d4[:, :, ROWS], fwd(d3), dj,
                               Alu.bitwise_and, d3[:, :, ROWS])
            yield lambda: _stt(V, a2, bwd(d2), -dj, Alu.bitwise_or,
                               d2[:, :, ROWS])
            yield lambda: _stt(V, t3[:, :, ROWS], bwd(d3), -dj,
                               Alu.bitwise_or, d3[:, :, ROWS])
            yield lambda: _stt(V, a3, bwd(d3, 2), -2 * dj, Alu.bitwise_or,
                               t3[:, :, ROWS])
            yield lambda: _stt(V, t4[:, :, ROWS], bwd(d4), -dj,
                               Alu.bitwise_or, d4[:, :, ROWS])
            yield lambda: _stt(V, a4, bwd(t4, 2), -2 * dj, Alu.bitwise_or,
                               t4[:, :, ROWS])

        for ka, kb in ((0, 1), (2, 3)):
            for opa, opb in zip(conn_ops(0, ka), conn_ops(1, kb)):
                opa()
                opb()

        # conn merges
        V.tensor_tensor(cx1[:], AV[:, 0], AV[:, 3], Alu.bitwise_and)
        V.tensor_tensor(cx2[:], AV[:, 6], AV[:, 9], Alu.bitwise_and)
        V.tensor_tensor(cx1[:], cx1[:], cx2[:], Alu.bitwise_and)
        V.tensor_tensor(Rg[:, 0], mv, cx1[:], Alu.bitwise_xor)
        for kk, N in ((1, 2), (2, 3)):
            i0 = N - 2
            x4 = AV[:, i0::3]
            y4 = AV[:, i0 + 1::3]
            V.tensor_tensor(x4, x4, y4, Alu.bitwise_xor)
            V.tensor_tensor(cx1[:], x4[:, 0], x4[:, 1], Alu.bitwise_or)
            V.tensor_tensor(cx2[:], x4[:, 2], x4[:, 3], Alu.bitwise_or)
            V.tensor_tensor(Rg[:, kk], cx1[:], cx2[:], Alu.bitwise_or)

        # ---------- conn extraction + conversion + DMA ----------
        for j in range(8):
            V.tensor_scalar(mkL[:, :, :, j::8], Rg[:, 0:3], j, 0x00010001,
                            op0=Alu.logical_shift_right,
                            op1=Alu.bitwise_and)
        for ci, n0 in enumerate(range(0, NB, CB)):
            hs = slice(n0, n0 + CB)
            outt = epool.tile([P, CB, 10, 64], DT.bfloat16, name="outt")
            V.tensor_scalar(
                outt[:, :, 0:3],
                mkL16[:, :, hs, 0::2].rearrange("p c n x -> p n c x"),
                0, None, Alu.not_equal)
            V.tensor_scalar(
                outt[:, :, 3:6],
                mkL16[:, :, hs, 1::2].rearrange("p c n x -> p n c x"),
                0, None, Alu.not_equal)
            eng = (nc.gpsimd, nc.sync, A, A)[ci]
            eng.dma_start(
                out_v[:, hs, 128:512],
                outt[:, :, 0:6].rearrange("p b c x -> p b (c x)"))


_NC_CACHE = None


def _build_nc():
    global _NC_CACHE
    if _NC_CACHE is not None:
        return _NC_CACHE
    nc = bacc.Bacc("TRN2", debug=False, enable_asserts=False)
    state_d = nc.dram_tensor("state", [BPC, 64], DT.int8, kind="ExternalInput").ap()
    out_d = nc.dram_tensor("out", [BPC, 18 * 64], DT.bfloat16, kind="ExternalOutput").ap()
    with tile.TileContext(nc) as tc:
        feature_kernel(tc, out_d, state_d)
    nc.finalize()
    _NC_CACHE = nc
    return nc


_JIT_CACHE = None


def _get_runner():
    """Build a jitted shard_map runner over the 8 cores, fed with
    pre-sharded jax Arrays (avoids XLA-side resharding programs, which the
    neuron compiler chokes on for these sizes)."""
    global _JIT_CACHE
    if _JIT_CACHE is not None:
        return _JIT_CACHE
    import jax
    from jax.sharding import Mesh, PartitionSpec, NamedSharding
    try:
        from jax.experimental.shard_map import shard_map
    except ImportError:
        from jax.shard_map import shard_map  # newer jax
    from concourse import bass2jax as B2J

    B2J.install_neuronx_cc_hook()
    nc = _build_nc()

    in_names = ["state"]
    out_names = ["out"]
    import ml_dtypes
    out_avals = [jax.core.ShapedArray((BPC, 18 * 64), ml_dtypes.bfloat16)]
    all_names = in_names + out_names
    if nc.partition_id_tensor is not None:
        all_names = all_names + [nc.partition_id_tensor.name]

    def _body(state_a, zeros_a):
        operands = [state_a, zeros_a]
        if nc.partition_id_tensor is not None:
            operands.append(B2J.partition_id_tensor())
        outs = B2J._bass_exec_p.bind(
            *operands,
            out_avals=tuple(out_avals),
            in_names=tuple(all_names),
            out_names=tuple(out_names),
            lowering_input_output_aliases=(),
            sim_require_finite=True,
            sim_require_nnan=True,
            nc=nc,
        )
        return outs[0]

    devices = jax.devices()[:NCORES]
    mesh = Mesh(np.asarray(devices), ("core",))
    spec = PartitionSpec("core")
    sharded = jax.jit(
        shard_map(
            _body, mesh=mesh,
            in_specs=(spec, spec),
            out_specs=spec,
            check_rep=False,
        ),
        donate_argnums=(1,),
        keep_unused=True,
    )

    def put(shards):
        arrs = [jax.device_put(s, devices[i]) for i, s in enumerate(shards)]
        global_shape = (sum(s.shape[0] for s in shards),) + shards[0].shape[1:]
        return jax.make_array_from_single_device_arrays(
            global_shape, NamedSharding(mesh, spec), arrs
        )

    _JIT_CACHE = (sharded, put)
    return _JIT_CACHE


def kernel(state, side):
    """Full-input entry point: state [32768,8,8] f32, side [32768] f32."""
    state = np.asarray(state, dtype=np.float32).reshape(-1, 64)
    side = np.asarray(side, dtype=np.float32).reshape(-1)
    state = np.ascontiguousarray(state * side[:, None]).astype(np.int8)
    B = state.shape[0]
    assert B == BPC * NCORES, (B, BPC * NCORES)
    sharded, put = _get_runner()
    state_g = put([state[i * BPC:(i + 1) * BPC] for i in range(NCORES)])
    import ml_dtypes
    zeros_g = put([np.zeros((BPC, 18 * 64), ml_dtypes.bfloat16) for _ in range(NCORES)])
    out = sharded(state_g, zeros_g)
    out = np.asarray(out).astype(np.float32).reshape(NCORES * BPC, 18, 8, 8)
    return out


# revision 28
# speedup vs baseline: 1.0105x; 1.0105x over previous
"""v4.1 feature kernel.

Measured-model driven layout:
- DVE (V) does all word algebra; ~0.52 ns/e TS, ~1.04 ns/e TT/STT, +~200ns/op.
- GpSimd unused for compute: Pool has no bitwise/TS support, and its memsets
  contend with DVE for SBUF ports (measured 3x slowdown on concurrent ops).
- All guard memsets run on V up front, inside the input-DMA idle window.
- Act does the strided u16->f32 expansions for the line group (2x stride
  penalty measured, still worth it: runs while V computes conn) plus the
  doubles broadcasts; V converts the conn group at stream end (V idle then).
- Extraction is j-major over all 32 boards (8 TS ops per 3-lane group).
- Conn group extracted+converted per 16-board half to shorten the DMA tail.
- TMP (line temps) and CV (conn pyramids) share one scratch allocation
  (disjoint lifetimes); mkL is reused for the conn extraction.
"""
import numpy as np

import concourse.bass as bass
import concourse.bacc as bacc
import concourse.mybir as mybir
import concourse.tile as tile

Alu = mybir.AluOpType
Act = mybir.ActivationFunctionType
DT = mybir.dt

P = 128
NB = 32
CB = 8             # boards per output chunk
NCORES = 8
BPC = P * NB
PAD = 18
R0 = 5
ROWS = slice(R0, R0 + 8)
SEG = 0x00FF00FF
PADT = 12
R0T = 2

DIRS = ((0, 1), (1, 0), (1, 1), (1, -1))

NAMES = ("t", "u", "a", "w", "b", "y", "q", "l2", "m3", "r1", "c",
         "i1", "l3", "lb", "d0", "d1", "d", "j1", "md", "o3", "rb",
         "x", "e", "g1", "g2", "r3", "c_l2", "c_l3", "c_r3")
NLANES = len(NAMES)
SCR_WORDS = NLANES * NB * PADT           # TMP scratch, u32 per partition
CV_WORDS = 2 * 5 * NB * PAD
assert CV_WORDS <= SCR_WORDS


def _stt_raw(eng, out, in0, imm, in1, op0, op1, imm_dt=DT.uint32):
    outs = [eng.lower_ap(out)]
    return eng.add_instruction(
        mybir.InstTensorScalarPtr(
            name=eng.bass.get_next_instruction_name(),
            is_scalar_tensor_tensor=True,
            op0=op0, op1=op1,
            ins=[eng.lower_ap(in0),
                 mybir.ImmediateValue(dtype=imm_dt, value=imm),
                 eng.lower_ap(in1)],
            outs=outs,
        )
    )


def _stt(eng, out, in0, sh, op1, in1):
    if sh > 0:
        _stt_raw(eng, out, in0, sh, in1, Alu.logical_shift_left, op1)
    elif sh < 0:
        _stt_raw(eng, out, in0, -sh, in1, Alu.logical_shift_right, op1)
    else:
        eng.tensor_tensor(out, in0, in1, op1)


def feature_kernel(tc, out_d, state_d):
    nc = tc.nc
    V, A = nc.vector, nc.scalar

    state_v = state_d.rearrange("(p n) c -> p n c", p=P)
    out_v = out_d.rearrange("(p n) c -> p n c", p=P)

    with (
        tc.tile_pool(name="main", bufs=1) as pool,
        tc.tile_pool(name="exp", bufs=4) as epool,
    ):
        myR = pool.tile([P, NB, 8], DT.uint32, name="myR")
        opR = pool.tile([P, NB, 8], DT.uint32, name="opR")
        P4 = pool.tile([P, 4, NB, PAD], DT.uint32, name="P4")
        Rg = pool.tile([P, 6, NB, 8], DT.uint32, name="Rg")
        dge = pool.tile([P, 4, NB], DT.float32, name="dge")
        lbm = pool.tile([P, NB, 8], DT.uint32, name="lbm")
        rbm = pool.tile([P, NB, 8], DT.uint32, name="rbm")
        mkL = pool.tile([P, 3, NB, 64], DT.uint32, name="mkL")
        w1 = pool.tile([P, 3, NB, 8], DT.uint32, name="w1")
        w2 = pool.tile([P, 3, NB, 8], DT.uint32, name="w2")
        cs23 = pool.tile([P, 4, NB], DT.uint32, name="cs23")
        scratch = pool.tile([P, SCR_WORDS], DT.uint32, name="scratch")
        AV = pool.tile([P, 12, NB, 8], DT.uint32, name="AV")
        cx1 = pool.tile([P, NB, 8], DT.uint32, name="cx1")
        cx2 = pool.tile([P, NB, 8], DT.uint32, name="cx2")

        TMP = scratch.rearrange("p (l n c) -> p l n c", l=NLANES, n=NB)
        CV = scratch[:, 0:CV_WORDS].rearrange(
            "p (w f n c) -> p w f n c", w=2, f=5, n=NB)
        nix = {n: i for i, n in enumerate(NAMES)}

        with tc.tile_pool(name="pre", bufs=1) as ppre:
            s8 = ppre.tile([P, NB, 64], DT.int8, name="s8")
            nc.sync.dma_start(s8[:], state_v)
            mo = ppre.tile([P, NB, 2, 64], DT.bfloat16, name="mo")
            pk1 = ppre.tile([P, NB, 8, 4], DT.float32, name="pk1")
            pk2 = ppre.tile([P, NB, 8, 2], DT.float32, name="pk2")

            A.activation(mo[:, :, 0], s8[:], Act.Relu)
            A.activation(mo[:, :, 1], s8[:], Act.Relu, 0.0, -1.0)
            V.memset(P4[:, :, :, 0:R0], 0)
            V.memset(P4[:, :, :, R0 + 8:PAD], 0)
            V.memset(lbm[:], 0)
            V.memset(lbm[:, :, 0:1], SEG)
            V.memset(rbm[:], 0)
            V.memset(rbm[:, :, 3:8], SEG)
            nc.sync.dma_start(
                out_v[:, :, 0:128],
                mo.rearrange("p n c x -> p n (c x)"))

            def pack(dst_ap, srcf):
                v = srcf.rearrange("p n (r j2 t) -> p n r j2 t", t=2, j2=4)
                a1, b1 = v[:, :, :, :, 1], v[:, :, :, :, 0]
                V.scalar_tensor_tensor(pk1[:], a1, 2.0, b1, op0=Alu.mult,
                                       op1=Alu.add)
                ww2 = pk1.rearrange("p n r (k t) -> p n r k t", t=2)
                a2, b2 = ww2[:, :, :, :, 1], ww2[:, :, :, :, 0]
                V.scalar_tensor_tensor(pk2[:], a2, 4.0, b2, op0=Alu.mult,
                                       op1=Alu.add)
                a3, b3 = pk2[:, :, :, 1], pk2[:, :, :, 0]
                V.scalar_tensor_tensor(dst_ap, a3, 16.0, b3, op0=Alu.mult,
                                       op1=Alu.add)

            pack(myR[:], mo[:, :, 0])
            pack(opR[:], mo[:, :, 1])

        # ---------- planes ----------
        Ad, Bd, Ed, Nd = (P4[:, i] for i in range(4))
        _stt(V, Ad[:, :, ROWS], opR[:], 16, Alu.bitwise_or, myR[:])
        _stt(V, Bd[:, :, ROWS], myR[:], 16, Alu.bitwise_or, opR[:])
        V.tensor_tensor(Ed[:, :, ROWS], Ad[:, :, ROWS], Bd[:, :, ROWS],
                        Alu.bitwise_or)
        V.tensor_scalar(Ed[:, :, ROWS], Ed[:, :, ROWS], SEG, None,
                        Alu.bitwise_xor)
        V.tensor_scalar(Nd[:, :, ROWS], Ad[:, :, ROWS], SEG, None,
                        Alu.bitwise_xor)

        V.memset(TMP[:, :, :, 0:R0T], 0)
        V.memset(TMP[:, :, :, R0T + 8:PADT], 0)

        def TT(n, k=0):
            return TMP[:, nix[n], :, R0T + k:R0T + 8 + k]

        # ---------- line features: rows (3 sub-chains zipped) ----------
        me, op, em, nm = (x[:, :, ROWS] for x in (Ad, Bd, Ed, Nd))
        T = TT
        _stt(V, T("t"), me, -1, Alu.bitwise_and, me)
        _stt(V, T("u"), em, -1, Alu.bitwise_and, em)
        V.tensor_scalar(T("lb"), op, 1, 0x00010001,
                        op0=Alu.logical_shift_left, op1=Alu.bitwise_or)
        _stt(V, T("m3"), me, -2, Alu.bitwise_and, T("t"))
        _stt(V, T("a"), T("u"), -2, Alu.bitwise_and, T("t"))
        _stt(V, T("w"), em, -3, Alu.bitwise_and, em)
        V.tensor_scalar(T("rb"), op, 5, 0x00F800F8,
                        op0=Alu.logical_shift_right, op1=Alu.bitwise_or)
        _stt(V, T("r1"), em, -4, Alu.bitwise_and, em)
        _stt(V, T("b"), T("t"), -1, Alu.bitwise_and, T("w"))
        _stt(V, T("d0"), em, -3, Alu.bitwise_and, T("m3"))
        _stt(V, T("c"), T("m3"), -1, Alu.bitwise_and, T("r1"))
        _stt(V, T("y"), T("b"), 1, Alu.bitwise_or, T("b"))
        _stt(V, T("d1"), nm, -4, Alu.bitwise_and, T("d0"))
        _stt(V, T("i1"), T("c"), 1, Alu.bitwise_or, T("c"))
        _stt(V, T("o3"), T("m3"), -1, Alu.bitwise_and, nm)
        V.tensor_tensor(T("q"), T("a"), T("y"), Alu.bitwise_or)
        V.tensor_tensor(T("d"), T("d1"), T("lb"), Alu.bitwise_and)
        _stt(V, T("l3"), T("i1"), 1, Alu.bitwise_or, T("c"))
        _stt(V, T("o3"), nm, -4, Alu.bitwise_and, T("o3"))
        _stt(V, T("l2"), T("q"), 1, Alu.bitwise_or, T("a"))
        _stt(V, T("j1"), T("d"), 1, Alu.bitwise_or, T("d"))
        V.tensor_tensor(T("x"), T("lb"), T("rb"), Alu.bitwise_xor)
        _stt(V, T("md"), T("d"), 2, Alu.bitwise_or, T("j1"))
        V.tensor_tensor(T("e"), T("o3"), T("x"), Alu.bitwise_and)
        _stt(V, T("g1"), T("e"), 1, Alu.bitwise_or, T("e"))
        _stt(V, T("g2"), T("g1"), 1, Alu.bitwise_or, T("e"))
        _stt(V, T("r3"), T("g2"), 1, Alu.bitwise_or, T("md"))

        # ---------- line features: cols (offset addressing, zipped) ----
        def dn(x, k):
            return x[:, :, R0 + k:R0 + 8 + k]

        def Tc(n, k=0):
            nm2 = "c_" + n if n in ("l2", "l3", "r3") else n
            return TT(nm2, k)

        def MV(x, k=0):
            return x[:, :, R0 + k:R0 + 8 + k]

        me4, op4, em4, nm4 = Ad, Bd, Ed, Nd
        V.tensor_tensor(Tc("t"), MV(me4), dn(me4, 1), Alu.bitwise_and)
        V.tensor_tensor(Tc("u"), MV(em4), dn(em4, 1), Alu.bitwise_and)
        V.tensor_tensor(Tc("lb"), MV(op4, -1), lbm[:], Alu.bitwise_or)
        V.tensor_tensor(Tc("m3"), Tc("t"), dn(me4, 2), Alu.bitwise_and)
        V.tensor_tensor(Tc("a"), Tc("t"), Tc("u", 2), Alu.bitwise_and)
        V.tensor_tensor(Tc("w"), MV(em4), dn(em4, 3), Alu.bitwise_and)
        V.tensor_tensor(Tc("rb"), MV(op4, 5), rbm[:], Alu.bitwise_or)
        V.tensor_tensor(Tc("r1"), MV(em4), dn(em4, 4), Alu.bitwise_and)
        V.tensor_tensor(Tc("b"), Tc("w"), Tc("t", 1), Alu.bitwise_and)
        V.tensor_tensor(Tc("d0"), Tc("m3"), dn(em4, 3), Alu.bitwise_and)
        V.tensor_tensor(Tc("c"), Tc("r1"), Tc("m3", 1), Alu.bitwise_and)
        V.tensor_tensor(Tc("y"), Tc("b"), Tc("b", -1), Alu.bitwise_or)
        V.tensor_tensor(Tc("d1"), Tc("d0"), dn(nm4, 4), Alu.bitwise_and)
        V.tensor_tensor(Tc("i1"), Tc("c"), Tc("c", -1), Alu.bitwise_or)
        V.tensor_tensor(Tc("o3"), Tc("m3", 1), MV(nm4), Alu.bitwise_and)
        V.tensor_tensor(Tc("q"), Tc("a"), Tc("y"), Alu.bitwise_or)
        V.tensor_tensor(Tc("d"), Tc("d1"), Tc("lb"), Alu.bitwise_and)
        V.tensor_tensor(Tc("l3"), Tc("c"), Tc("i1", -1), Alu.bitwise_or)
        V.tensor_tensor(Tc("o3"), Tc("o3"), dn(nm4, 4), Alu.bitwise_and)
        V.tensor_tensor(Tc("l2"), Tc("a"), Tc("q", -1), Alu.bitwise_or)
        V.tensor_tensor(Tc("j1"), Tc("d"), Tc("d", -1), Alu.bitwise_or)
        V.tensor_tensor(Tc("x"), Tc("lb"), Tc("rb"), Alu.bitwise_xor)
        V.tensor_tensor(Tc("md"), Tc("j1"), Tc("d", -2), Alu.bitwise_or)
        V.tensor_tensor(Tc("e"), Tc("o3"), Tc("x"), Alu.bitwise_and)
        V.tensor_tensor(Tc("g1"), Tc("e"), Tc("e", -1), Alu.bitwise_or)
        V.tensor_tensor(Tc("g2"), Tc("e"), Tc("g1", -1), Alu.bitwise_or)
        V.tensor_tensor(Tc("r3"), Tc("md"), Tc("g2", -1), Alu.bitwise_or)

        V.tensor_tensor(Rg[:, 3], TT("l2"), TT("c_l2"), Alu.bitwise_or)
        _stt(V, Rg[:, 4], TT("l3"), 1, Alu.bitwise_or, TT("c_l3", -1))
        V.tensor_tensor(Rg[:, 5], TT("r3"), TT("c_r3"), Alu.bitwise_or)

        # ---------- doubles (SWAR popcount, all V) ----------
        V.tensor_scalar(w1[:], Rg[:, 3:6], 1, 0x00550055,
                        op0=Alu.logical_shift_right, op1=Alu.bitwise_and)
        V.tensor_tensor(w1[:], Rg[:, 3:6], w1[:], Alu.subtract)
        V.tensor_scalar(w2[:], w1[:], 2, 0x00330033,
                        op0=Alu.logical_shift_right, op1=Alu.bitwise_and)
        V.tensor_scalar(w1[:], w1[:], 0x00330033, None, Alu.bitwise_and)
        V.tensor_tensor(w1[:], w1[:], w2[:], Alu.add)
        V.tensor_scalar(w2[:], w1[:], 4, None, Alu.logical_shift_right)
        V.tensor_tensor(w1[:], w1[:], w2[:], Alu.add)
        V.tensor_scalar(w1[:], w1[:], 0x000F000F, None, Alu.bitwise_and)
        V.tensor_tensor(w1[:, :, :, 0:4], w1[:, :, :, 0:4],
                        w1[:, :, :, 4:8], Alu.add)
        V.tensor_tensor(w1[:, :, :, 0:2], w1[:, :, :, 0:2],
                        w1[:, :, :, 2:4], Alu.add)
        V.tensor_tensor(cs23[:, 0:3], w1[:, :, :, 0], w1[:, :, :, 1],
                        Alu.add)
        V.tensor_tensor(cs23[:, 3], cs23[:, 1], cs23[:, 2], Alu.add)
        V.tensor_scalar(cs23[:, 1], cs23[:, 0], 0xFF, None, Alu.bitwise_and)
        V.tensor_scalar(cs23[:, 2], cs23[:, 3], 0xFF, None, Alu.bitwise_and)
        V.tensor_scalar(dge[:, 0], cs23[:, 1], 2, None, Alu.is_ge)
        V.tensor_scalar(dge[:, 1], cs23[:, 2], 2, None, Alu.is_ge)
        V.tensor_scalar(dge[:, 2], cs23[:, 0], 2 << 16, None, Alu.is_ge)
        V.tensor_scalar(dge[:, 3], cs23[:, 3], 2 << 16, None, Alu.is_ge)

        # ---------- line-group extraction (j-major, all boards) ----------
        for j in range(8):
            V.tensor_scalar(mkL[:, :, :, j::8], Rg[:, 3:6], j, 0x00010001,
                            op0=Alu.logical_shift_right, op1=Alu.bitwise_and)
        mkL16 = mkL[:].bitcast(DT.uint16)   # [P, 3, NB, 128]

        # ---------- line-group conversion + DMA ----------
        # chunks 0-2 convert on Act (runs while V computes conn);
        # chunk 3 converts on V (then V moves to conn).
        line_outts = []
        for ci, n0 in enumerate(range(0, NB, CB)):
            hs = slice(n0, n0 + CB)
            outt = epool.tile([P, CB, 10, 64], DT.bfloat16, name="outt")
            line_outts.append((hs, outt))
            A.activation(
                outt[:, :, 0:3],
                mkL16[:, :, hs, 0::2].rearrange("p c n x -> p n c x"),
                Act.Copy)
            A.activation(
                outt[:, :, 5:8],
                mkL16[:, :, hs, 1::2].rearrange("p c n x -> p n c x"),
                Act.Copy)
            A.activation(
                outt[:, :, 3:5],
                dge[:, 0:2, hs].rearrange("p d n -> p n d")[:, :, :, None]
                .broadcast_to((P, CB, 2, 64)), Act.Copy)
            A.activation(
                outt[:, :, 8:10],
                dge[:, 2:4, hs].rearrange("p d n -> p n d")[:, :, :, None]
                .broadcast_to((P, CB, 2, 64)), Act.Copy)
            nc.sync.dma_start(
                out_v[:, hs, 512:1152],
                outt.rearrange("p b c x -> p b (c x)"))

        # ---------- connectivity (2-dir interleaved chains) ----------
        V.memset(CV[:, :, :, :, R0 - 1:R0], 0)
        V.memset(CV[:, :, :, :, R0 + 8:R0 + 10], 0)

        mv = Ad[:, :, ROWS]

        def conn_ops(w, k):
            di, dj = DIRS[k]
            d2, d3, d4, t3, t4 = (CV[:, w, i] for i in range(5))

            def fwd(t):
                return t[:, :, R0 - di:R0 + 8 - di]

            def bwd(t, m=1):
                return t[:, :, R0 + m * di:R0 + 8 + m * di]

            a2, a3, a4 = (AV[:, 3 * k + i] for i in range(3))
            yield lambda: _stt(V, d2[:, :, ROWS], fwd(Ad), dj,
                               Alu.bitwise_and, mv)
            yield lambda: _stt(V, d3[:, :, ROWS], fwd(d2), dj,
                               Alu.bitwise_and, d2[:, :, ROWS])
            yield lambda: _stt(V, d4[:, :, ROWS], fwd(d3), dj,
                               Alu.bitwise_and, d3[:, :, ROWS])
            yield lambda: _stt(V, a2, bwd(d2), -dj, Alu.bitwise_or,
                               d2[:, :, ROWS])
            yield lambda: _stt(V, t3[:, :, ROWS], bwd(d3), -dj,
                               Alu.bitwise_or, d3[:, :, ROWS])
            yield lambda: _stt(V, a3, bwd(d3, 2), -2 * dj, Alu.bitwise_or,
                               t3[:, :, ROWS])
            yield lambda: _stt(V, t4[:, :, ROWS], bwd(d4), -dj,
                               Alu.bitwise_or, d4[:, :, ROWS])
            yield lambda: _stt(V, a4, bwd(t4, 2), -2 * dj, Alu.bitwise_or,
                               t4[:, :, ROWS])

        for ka, kb in ((0, 1), (2, 3)):
            for opa, opb in zip(conn_ops(0, ka), conn_ops(1, kb)):
                opa()
                opb()

        # conn merges
        V.tensor_tensor(cx1[:], AV[:, 0], AV[:, 3], Alu.bitwise_and)
        V.tensor_tensor(cx2[:], AV[:, 6], AV[:, 9], Alu.bitwise_and)
        V.tensor_tensor(cx1[:], cx1[:], cx2[:], Alu.bitwise_and)
        V.tensor_tensor(Rg[:, 0], mv, cx1[:], Alu.bitwise_xor)
        for kk, N in ((1, 2), (2, 3)):
            i0 = N - 2
            x4 = AV[:, i0::3]
            y4 = AV[:, i0 + 1::3]
            V.tensor_tensor(x4, x4, y4, Alu.bitwise_xor)
            V.tensor_tensor(cx1[:], x4[:, 0], x4[:, 1], Alu.bitwise_or)
            V.tensor_tensor(cx2[:], x4[:, 2], x4[:, 3], Alu.bitwise_or)
            V.tensor_tensor(Rg[:, kk], cx1[:], cx2[:], Alu.bitwise_or)

        # ---------- conn extraction + conversion + DMA ----------
        for j in range(8):
            V.tensor_scalar(mkL[:, :, :, j::8], Rg[:, 0:3], j, 0x00010001,
                            op0=Alu.logical_shift_right,
                            op1=Alu.bitwise_and)
        for ci, n0 in enumerate(range(0, NB, CB)):
            hs = slice(n0, n0 + CB)
            outt = epool.tile([P, CB, 10, 64], DT.bfloat16, name="outt")
            V.tensor_scalar(
                outt[:, :, 0:3],
                mkL16[:, :, hs, 0::2].rearrange("p c n x -> p n c x"),
                0, None, Alu.not_equal)
            V.tensor_scalar(
                outt[:, :, 3:6],
                mkL16[:, :, hs, 1::2].rearrange("p c n x -> p n c x"),
                0, None, Alu.not_equal)
            eng = (A, nc.sync, A, nc.gpsimd)[ci]
            eng.dma_start(
                out_v[:, hs, 128:512],
                outt[:, :, 0:6].rearrange("p b c x -> p b (c x)"))


_NC_CACHE = None


def _build_nc():
    global _NC_CACHE
    if _NC_CACHE is not None:
        return _NC_CACHE
    nc = bacc.Bacc("TRN2", debug=False, enable_asserts=False)
    state_d = nc.dram_tensor("state", [BPC, 64], DT.int8, kind="ExternalInput").ap()
    out_d = nc.dram_tensor("out", [BPC, 18 * 64], DT.bfloat16, kind="ExternalOutput").ap()
    with tile.TileContext(nc) as tc:
        feature_kernel(tc, out_d, state_d)
    nc.finalize()
    _NC_CACHE = nc
    return nc


_JIT_CACHE = None


def _get_runner():
    """Build a jitted shard_map runner over the 8 cores, fed with
    pre-sharded jax Arrays (avoids XLA-side resharding programs, which the
    neuron compiler chokes on for these sizes)."""
    global _JIT_CACHE
    if _JIT_CACHE is not None:
        return _JIT_CACHE
    import jax
    from jax.sharding import Mesh, PartitionSpec, NamedSharding
    try:
        from jax.experimental.shard_map import shard_map
    except ImportError:
        from jax.shard_map import shard_map  # newer jax
    from concourse import bass2jax as B2J

    B2J.install_neuronx_cc_hook()
    nc = _build_nc()

    in_names = ["state"]
    out_names = ["out"]
    import ml_dtypes
    out_avals = [jax.core.ShapedArray((BPC, 18 * 64), ml_dtypes.bfloat16)]
    all_names = in_names + out_names
    if nc.partition_id_tensor is not None:
        all_names = all_names + [nc.partition_id_tensor.name]

    def _body(state_a, zeros_a):
        operands = [state_a, zeros_a]
        if nc.partition_id_tensor is not None:
            operands.append(B2J.partition_id_tensor())
        outs = B2J._bass_exec_p.bind(
            *operands,
            out_avals=tuple(out_avals),
            in_names=tuple(all_names),
            out_names=tuple(out_names),
            lowering_input_output_aliases=(),
            sim_require_finite=True,
            sim_require_nnan=True,
            nc=nc,
        )
        return outs[0]

    devices = jax.devices()[:NCORES]
    mesh = Mesh(np.asarray(devices), ("core",))
    spec = PartitionSpec("core")
    sharded = jax.jit(
        shard_map(
            _body, mesh=mesh,
            in_specs=(spec, spec),
            out_specs=spec,
            check_rep=False,
        ),
        donate_argnums=(1,),
        keep_unused=True,
    )

    def put(shards):
        arrs = [jax.device_put(s, devices[i]) for i, s in enumerate(shards)]
        global_shape = (sum(s.shape[0] for s in shards),) + shards[0].shape[1:]
        return jax.make_array_from_single_device_arrays(
            global_shape, NamedSharding(mesh, spec), arrs
        )

    _JIT_CACHE = (sharded, put)
    return _JIT_CACHE


def kernel(state, side):
    """Full-input entry point: state [32768,8,8] f32, side [32768] f32."""
    state = np.asarray(state, dtype=np.float32).reshape(-1, 64)
    side = np.asarray(side, dtype=np.float32).reshape(-1)
    state = np.ascontiguousarray(state * side[:, None]).astype(np.int8)
    B = state.shape[0]
    assert B == BPC * NCORES, (B, BPC * NCORES)
    sharded, put = _get_runner()
    state_g = put([state[i * BPC:(i + 1) * BPC] for i in range(NCORES)])
    import ml_dtypes
    zeros_g = put([np.zeros((BPC, 18 * 64), ml_dtypes.bfloat16) for _ in range(NCORES)])
    out = sharded(state_g, zeros_g)
    out = np.asarray(out).astype(np.float32).reshape(NCORES * BPC, 18, 8, 8)
    return out
